# revision 1
# baseline (speedup 1.0000x reference)
"""Trainium2 Bass kernel for ACLIP top-k patch masking.

Reference computation (per batch):
    cls, patches = split(image_features)            # [1,D], [P,D]  P=576
    sim = normalize(patches) @ normalize(text)      # [P]
    idx = sort(top_k(sim, 288).indices)             # [288]
    out = concat([cls, patches[idx]])               # [289, D]

Distribution: pure data parallel, batch 256 -> 32 per core x 8 cores.

Per-core algorithm (B=32 batches, P=576 patches, D=1024, K=288):
  - Load patch rows [128, 5, 1024] per batch (chunk 4 half-filled).
  - prod = X * text_bcast (text norm is a positive per-batch constant and
    cannot change the top-k ordering, so text is used unnormalized).
  - s[p] = sum_d prod[p, d], n[p] = sum_d X[p, d]^2.
  - u[p] = sign(s) * s^2 / n, a monotone transform of the cosine sim
    (avoids sqrt, which only exists on the scalar engine).
  - rank[p] = #{q: u[q] > u[p]} exactly, comparing each u-column against
    a PSUM row of all 576 sims built by a diag(u)-matmul broadcast.
    DVE chunks: tensor_scalar is_gt + accum. ACT chunks: Sign(u_p - u_q)
    + accum gives 575 - 2*rank, so keep (rank < K) becomes signsum >= 0.
  - dest slot = within-chunk cumsum (triangular matmul) + chunk offsets,
    accumulated into one PSUM tile. Kept rows are written by an indirect
    scatter DMA to rows [b*289+1 ...]; dropped rows get dest=1e6 and
    tie-overflow slots exceed the DMA bounds check, so both are skipped.
    CLS rows go by a strided DMA.

The per-batch chain is long (load -> sims -> rank -> scatter), so the
emission order is software-pipelined: stage A (loads + streaming sims)
for batch b is emitted alongside stage B (rank + scatter) for batch
b-LAG, which keeps every engine's in-order queue from serializing
consecutive batches.
"""

import numpy as np

import concourse.bass as bass
import concourse.mybir as mybir
import concourse.tile as tile
from concourse import bacc
from concourse.bass import IndirectOffsetOnAxis
from concourse.masks import make_identity, make_upper_triangular

F32 = mybir.dt.float32
I32 = mybir.dt.int32

B_FULL = 256
N_CORES = 8
B_CORE = B_FULL // N_CORES
NUM_TOKENS = 577
P = 576          # patches per batch
D = 1024
K = 288          # kept patches
OUT_TOK = K + 1  # cls + kept
NCH = 5          # 128-row chunks per batch (4 full + 1 of 64)
LAST = P - 4 * 128  # rows in last chunk = 64
# Skip sentinel for dropped rows. Must be f32-exact, > any valid row index,
# and small enough that sentinel * D stays within int32 (the indirect DMA
# multiplies indices by the row stride).
BIG = 1.0e6

RANK_ACT = (0, 1, 2)  # rank chunks on ACT (prefix of 0..3; never chunk 4)
LAG = 4            # stage-B emission lag (batches)


def _stage_a(nc, pools, img, txt, b):
    """Loads + streaming sims: prod, s, n."""
    (xpool, prpool, bcpool, spool, jpool, trpool, dgpool,
     pprow, pprbc, ppcum) = pools
    st = {}
    x = xpool.tile([128, NCH, D], F32, tag="x")
    nc.sync.dma_start(
        out=x[:, 0:4, :],
        in_=img[b, 1:513, :].rearrange("(c p) d -> p c d", p=128),
    )
    nc.sync.dma_start(out=x[0:LAST, 4, :], in_=img[b, 513:577, :])
    st["x"] = x

    txtb = bcpool.tile([128, D], F32, tag="txtb")
    nc.scalar.dma_start(out=txtb[0:1, :], in_=txt[b : b + 1, :])
    w = 1
    while w < 128:
        nc.scalar.dma_start(out=txtb[w : 2 * w, :], in_=txtb[0:w, :])
        w *= 2

    S = spool.tile([128, NCH], F32, tag="S")
    N = spool.tile([128, NCH], F32, tag="N")
    nc.vector.memset(S[LAST:128, 4:5], 0.0)
    nc.vector.memset(N[LAST:128, 4:5], 1.0)
    st["S"], st["N"] = S, N

    prod = prpool.tile([128, 4, D], F32, tag="prod")
    nc.vector.tensor_tensor(
        out=prod[:, :, :], in0=x[:, 0:4, :],
        in1=txtb[:, None, :].to_broadcast([128, 4, D]),
        op=mybir.AluOpType.mult,
    )
    nc.vector.tensor_reduce(
        out=S[:, 0:2], in_=prod[:, 0:2, :],
        axis=mybir.AxisListType.X, op=mybir.AluOpType.add,
    )
    for c in (2, 3):
        jsr = jpool.tile([128, D], F32, tag="ja")
        nc.scalar.activation(
            out=jsr[:, :], in_=prod[:, c, :],
            func=mybir.ActivationFunctionType.Copy,
            accum_out=S[:, c : c + 1],
        )
    prod4 = prpool.tile([128, D], F32, tag="prod4")
    nc.gpsimd.tensor_tensor(
        out=prod4[0:LAST, :], in0=x[0:LAST, 4, :],
        in1=txtb[0:LAST, :], op=mybir.AluOpType.mult,
    )
    ja = jpool.tile([128, D], F32, tag="ja")
    nc.scalar.activation(
        out=ja[0:LAST, :], in_=prod4[0:LAST, :],
        func=mybir.ActivationFunctionType.Copy,
        accum_out=S[0:LAST, 4:5],
    )
    for c in range(NCH):
        rows = 128 if c < 4 else LAST
        js = jpool.tile([128, D], F32, tag="ja")
        nc.scalar.activation(
            out=js[:rows, :], in_=x[:rows, c, :],
            func=mybir.ActivationFunctionType.Square,
            accum_out=N[:rows, c : c + 1],
        )
    return st


def _strip_out_waw(inst_h, prior_names):
    """Remove sync deps on earlier out-writers. All writers of `out` touch
    provably disjoint rows (CLS row 0; batch b's scatters only rows
    [b*289+1, b*289+288], slots unique via cumsum), so the completion-order
    WAW edges Tile inserts between them only serialize the DMA queue."""
    inst = getattr(inst_h, "ins", inst_h)
    for dep in list(inst.sync_dependency_names()):
        if dep in prior_names:
            try:
                inst.try_remove_dependency(dep)
            except Exception:
                inst.remove_dependency(dep)
    prior_names.add(inst.name)
    return inst


def _stage_b(nc, pools, consts, out_flat, b, st, out_writers):
    """u, ranks, destinations, scatter."""
    (xpool, prpool, bcpool, spool, jpool, trpool, dgpool,
     pprow, pprbc, ppcum) = pools
    ident, ltri, ones_col, ones_row, ones_mat = consts
    x, S, N = st["x"], st["S"], st["N"]

    # ---- u = sign(s) * s^2 / n  (monotone in the cosine sim) ----
    SS = spool.tile([128, NCH], F32, tag="SS")
    nc.vector.tensor_tensor(out=SS[:], in0=S[:], in1=S[:],
                            op=mybir.AluOpType.mult)
    REC = spool.tile([128, NCH], F32, tag="REC")
    nc.vector.reciprocal(REC[:], N[:])
    UA = spool.tile([128, NCH], F32, tag="UA")
    nc.vector.tensor_tensor(out=UA[:], in0=SS[:], in1=REC[:],
                            op=mybir.AluOpType.mult)
    SGN = spool.tile([128, NCH], I32, tag="SGN")
    nc.vector.tensor_scalar(
        out=SGN[:], in0=S[:].bitcast(I32), scalar1=-0x80000000,
        scalar2=None, op0=mybir.AluOpType.bitwise_and,
    )
    U = spool.tile([128, NCH], F32, tag="U")
    nc.vector.tensor_tensor(
        out=U[:].bitcast(I32), in0=UA[:].bitcast(I32), in1=SGN[:],
        op=mybir.AluOpType.bitwise_or,
    )
    # garbage rows of the half chunk must never rank into top-K
    nc.vector.memset(U[LAST:128, 4:5], -1e30)

    # ---- all-sims row in PSUM via one matmul per chunk:
    # rbc[p, j] = sum_k ones[k, p] * diag(u_col)[k, j] = u[j]
    rbcps = pprbc.tile([128, P], F32, tag="rbcps")
    for c in range(NCH):
        w = 128 if c < 4 else LAST
        diagU = dgpool.tile([128, 128], F32, tag="diagU")
        nc.vector.tensor_scalar(
            out=diagU[:], in0=ident[:],
            scalar1=U[:, c : c + 1], scalar2=None,
            op0=mybir.AluOpType.mult,
        )
        nc.tensor.matmul(
            rbcps[:, c * 128 : c * 128 + w],
            lhsT=ones_mat[:],
            rhs=diagU[:, 0:w],
            start=True, stop=True,
        )

    # ---- exact ranks ----
    RANK = spool.tile([128, NCH], F32, tag="RANK")
    for c in range(NCH):
        rows = 128 if c < 4 else LAST
        if c in RANK_ACT:
            jr = jpool.tile([128, P], F32, tag="jract")
            nc.scalar.activation(
                out=jr[:rows, :], in_=rbcps[:rows, :],
                func=mybir.ActivationFunctionType.Sign,
                bias=U[:rows, c : c + 1], scale=-1.0,
                accum_out=RANK[:rows, c : c + 1],
            )
        else:
            jr = jpool.tile([128, P], F32, tag="jrdve")
            nc.vector.tensor_scalar(
                out=jr[:rows, :], in0=rbcps[:rows, :],
                scalar1=U[:rows, c : c + 1], scalar2=0.0,
                op0=mybir.AluOpType.is_gt,
                op1=mybir.AluOpType.add,
                accum_out=RANK[:rows, c : c + 1],
            )
    nc.vector.memset(RANK[LAST:128, 4:5], 1e9)

    # ---- keep mask ----
    mask = spool.tile([128, NCH], F32, tag="mask")
    na = len(RANK_ACT)
    if na:
        nc.vector.tensor_scalar(
            out=mask[:, 0:na], in0=RANK[:, 0:na],
            scalar1=0.0, scalar2=None, op0=mybir.AluOpType.is_ge,
        )
    nc.vector.tensor_scalar(
        out=mask[:, na:NCH], in0=RANK[:, na:NCH],
        scalar1=float(K), scalar2=None, op0=mybir.AluOpType.is_lt,
    )

    # exclusive per-chunk offsets from the chunk totals
    tpsum = pprow.tile([1, NCH], F32, tag="rpsum")
    nc.tensor.matmul(tpsum[:], lhsT=ones_col[:], rhs=mask[:],
                     start=True, stop=True)
    tot = spool.tile([1, NCH], F32, tag="tot")
    nc.vector.tensor_copy(tot[:], tpsum[:])
    oinc = spool.tile([1, NCH], F32, tag="oinc")
    nc.vector.tensor_tensor_scan(
        out=oinc[:], data0=tot[:], data1=tot[:], initial=0.0,
        op0=mybir.AluOpType.add, op1=mybir.AluOpType.bypass,
    )
    offx = spool.tile([1, NCH], F32, tag="offx")
    nc.vector.tensor_tensor(out=offx[:], in0=oinc[:], in1=tot[:],
                            op=mybir.AluOpType.subtract)

    # G = within-chunk cumsum + chunk offset, accumulated in PSUM
    cpsum = ppcum.tile([128, NCH], F32, tag="cpsum")
    nc.tensor.matmul(cpsum[:], lhsT=ltri[:], rhs=mask[:],
                     start=True, stop=False)
    nc.tensor.matmul(cpsum[:], lhsT=ones_row[:], rhs=offx[:1, :],
                     start=False, stop=True)

    # dest = mask ? G + b*289 : BIG (tie overflow handled by bounds check)
    W = spool.tile([128, NCH], F32, tag="W")
    nc.vector.tensor_scalar(
        out=W[:], in0=mask[:], scalar1=-BIG,
        scalar2=BIG + float(b * OUT_TOK),
        op0=mybir.AluOpType.mult, op1=mybir.AluOpType.add,
    )
    DF = spool.tile([128, NCH], F32, tag="DF")
    nc.vector.tensor_tensor(out=DF[:], in0=cpsum[:], in1=W[:],
                            op=mybir.AluOpType.add)
    desti = spool.tile([128, NCH], I32, tag="desti")
    nc.vector.tensor_copy(out=desti[:], in_=DF[:])

    # ---- scatter kept rows (one offset column per chunk) ----
    for c in range(NCH):
        rows = 128 if c < 4 else LAST
        h = nc.gpsimd.indirect_dma_start(
            out=out_flat[:, :],
            out_offset=IndirectOffsetOnAxis(
                ap=desti[0:rows, c : c + 1], axis=0
            ),
            in_=x[0:rows, c, :],
            in_offset=None,
            bounds_check=b * OUT_TOK + K,
            oob_is_err=False,
        )
        _strip_out_waw(h, out_writers)


def build(nc, b_core=B_CORE, img=None, txt=None, out=None):
    if img is None:
        img = nc.dram_tensor("image_features", [b_core, NUM_TOKENS, D], F32,
                             kind="ExternalInput").ap()
        txt = nc.dram_tensor("text_features", [b_core, D], F32,
                             kind="ExternalInput").ap()
        out = nc.dram_tensor("out", [b_core, OUT_TOK, D], F32,
                             kind="ExternalOutput").ap()

    out_flat = out.rearrange("b k d -> (b k) d")

    with tile.TileContext(nc) as tc:
        with (
            tc.tile_pool(name="consts", bufs=1) as cpool,
            tc.tile_pool(name="x", bufs=5) as xpool,
            tc.tile_pool(name="prod", bufs=2) as prpool,
            tc.tile_pool(name="bcast", bufs=3) as bcpool,
            tc.tile_pool(name="small", bufs=8) as spool,
            tc.tile_pool(name="junk", bufs=3) as jpool,
            tc.tile_pool(name="trow", bufs=3) as trpool,
            tc.tile_pool(name="diag", bufs=4) as dgpool,
            tc.tile_pool(name="ps_row", bufs=2, space="PSUM") as pprow,
            tc.tile_pool(name="ps_rbc", bufs=2, space="PSUM") as pprbc,
            tc.tile_pool(name="ps_cum", bufs=2, space="PSUM") as ppcum,
        ):
            ident = cpool.tile([128, 128], F32)
            make_identity(nc, ident[:])
            ltri = cpool.tile([128, 128], F32)
            make_upper_triangular(nc, ltri[:], val=1.0, diag=True)
            ones_col = cpool.tile([128, 1], F32)
            nc.vector.memset(ones_col[:], 1.0)
            ones_row = cpool.tile([1, 128], F32)
            nc.vector.memset(ones_row[:], 1.0)
            ones_mat = cpool.tile([128, 128], F32)
            nc.vector.memset(ones_mat[:], 1.0)

            # CLS passthrough for all batches (SBUF bounce).
            out_writers = set()
            clsbuf = cpool.tile([b_core, D], F32)
            nc.sync.dma_start(out=clsbuf[:], in_=img[:, 0, :])
            hcls = nc.sync.dma_start(out=out[:, 0, :], in_=clsbuf[:])
            out_writers.add(getattr(hcls, "ins", hcls).name)

            pools = (xpool, prpool, bcpool, spool, jpool, trpool, dgpool,
                     pprow, pprbc, ppcum)
            consts = (ident, ltri, ones_col, ones_row, ones_mat)
            states = {}
            for i in range(b_core + LAG):
                if i < b_core:
                    states[i] = _stage_a(nc, pools, img, txt, i)
                j = i - LAG
                if j >= 0:
                    _stage_b(nc, pools, consts, out_flat, j, states.pop(j),
                             out_writers)
    return nc


_CACHED = {}


def _get_nc():
    if "nc" not in _CACHED:
        nc = bacc.Bacc("TRN2", target_bir_lowering=False)
        build(nc)
        nc.compile()
        _CACHED["nc"] = nc
    return _CACHED["nc"]


LAST_RESULT = None


def kernel(image_features, text_features):
    global LAST_RESULT
    from concourse.bass_utils import run_bass_kernel_spmd

    img = np.ascontiguousarray(np.asarray(image_features, dtype=np.float32))
    txt = np.ascontiguousarray(np.asarray(text_features, dtype=np.float32))
    assert img.shape == (B_FULL, NUM_TOKENS, D)
    assert txt.shape == (B_FULL, D)

    nc = _get_nc()
    in_maps = [
        {
            "image_features": img[i * B_CORE : (i + 1) * B_CORE],
            "text_features": txt[i * B_CORE : (i + 1) * B_CORE],
        }
        for i in range(N_CORES)
    ]
    res = run_bass_kernel_spmd(nc, in_maps, core_ids=list(range(N_CORES)))
    LAST_RESULT = res
    return np.concatenate([res.results[i]["out"] for i in range(N_CORES)], axis=0)



# revision 16
# speedup vs baseline: 1.4185x; 1.4185x over previous
"""Trainium2 Bass kernel for ACLIP top-k patch masking.

Reference computation (per batch):
    cls, patches = split(image_features)            # [1,D], [P,D]  P=576
    sim = normalize(patches) @ normalize(text)      # [P]
    idx = sort(top_k(sim, 288).indices)             # [288]
    out = concat([cls, patches[idx]])               # [289, D]

Distribution: pure data parallel, batch 256 -> 32 per core x 8 cores.

Per-core algorithm, two batches per tile ("pair" i -> batches 2i, 2i+1):
  - x[p, c, :] holds patch row 9*(p%64)+c of batch A (p<64) or B (p>=64):
    1152 rows pack a [128, 9, D] tile exactly, and each partition's 9 rows
    are contiguous in HBM (36KB descriptors).
  - text per pair is broadcast by one matmul (lhsT = half-selector [2,128],
    rhs = the two text rows) -> PSUM, bounced to SBUF by the scalar engine.
    Text norm is a positive per-batch constant that cannot change the
    per-batch top-k ordering, so text is used unnormalized.
  - One fused DVE op per chunk: prod = x * t with accum_out S[p,c]; one
    fused scalar/gpsimd op per chunk: x^2 with accum_out N[p,c].
  - u = sign(s) * s^2 / n, a monotone transform of the cosine sim.
  - RB[i, c', j] = u of the row at (partition j + 64*(i>=64), chunk c'):
    built by 9 matmuls lhsT=H (half indicator), rhs = D2 * u-column, where
    D2 = vstack(I64, I64) places both halves' values in 64 columns. Each
    row of RB sees exactly its own batch's 576 u values.
  - rank[p,c] = #{own batch q: u_q > u_pc} by 9 fused is_gt+accum DVE ops.
    keep = rank < 288.
  - dest slot = batch_base + (kept rows in lower partitions of own half,
    via a block-strict-triangular matmul) + (within-partition inclusive
    scan); dropped rows get +1e6 and are skipped by the DMA bounds check.
  - One indirect scatter DMA per pair writes all 1152 rows (offset AP
    [128, 9]); out-of-bounds destinations (dropped rows) are skipped.
    CLS rows go by a strided DMA once for all batches.

Stage A (loads + sims) for pair i is emitted alongside stage B
(rank + scatter) for pair i-LAG to keep the in-order engine queues busy.
"""

import numpy as np

import concourse.bass as bass
import concourse.mybir as mybir
import concourse.tile as tile
from concourse import bacc
from concourse.bass import IndirectOffsetOnAxis
from concourse.masks import make_identity, make_upper_triangular

F32 = mybir.dt.float32
I32 = mybir.dt.int32

B_FULL = 256
N_CORES = 8
B_CORE = B_FULL // N_CORES
NUM_TOKENS = 577
P = 576          # patches per batch
D = 1024
K = 288          # kept patches
OUT_TOK = K + 1  # cls + kept
NCH = 9          # chunks per pair tile (2 batches x 576 = 128 x 9)
NPAIR = B_CORE // 2
H = 64           # partitions per batch half
# Skip sentinel for dropped rows. Must be f32-exact, > any valid row index,
# and small enough that sentinel * D stays within int32 (the indirect DMA
# multiplies indices by the row stride).
BIG = 1.0e6

N_ACT = 5        # square chunks on the scalar engine (rest on gpsimd)
LAG = 2          # stage-B emission lag (pairs)


def _stage_a(nc, pools, consts, img, txt, i):
    """Loads + text broadcast + fused sims/norms for pair i."""
    b0 = 2 * i
    st = {}
    x = pools["x"].tile([128, NCH, D], F32, tag="x")
    nc.sync.dma_start(
        out=x[0:H, :, :],
        in_=img[b0, 1:NUM_TOKENS, :].rearrange("(p c) d -> p c d", c=NCH),
    )
    nc.sync.dma_start(
        out=x[H:128, :, :],
        in_=img[b0 + 1, 1:NUM_TOKENS, :].rearrange("(p c) d -> p c d", c=NCH),
    )
    st["x"] = x

    # The pair's two text rows (base partition 0, as the PE rhs requires).
    txtp = pools["txt"].tile([2, D], F32, tag="txtp")
    nc.sync.dma_start(out=txtp[:], in_=txt[b0 : b0 + 2, :])
    # txtbM[p, :] = text of batch A (p<64) / B (p>=64), via two matmuls
    # (one per 512-column PSUM bank: a matmul cannot span banks).
    txtbM = pools["ppt"].tile([128, D], F32, tag="txtbM")
    for h0 in (0, D // 2):
        nc.tensor.matmul(
            txtbM[:, h0 : h0 + D // 2], lhsT=consts["HS"][0:2, :],
            rhs=txtp[:, h0 : h0 + D // 2],
            start=True, stop=True,
        )
    txtbS = pools["act"].tile([128, D], F32, tag="txtbS")
    nc.scalar.activation(
        out=txtbS[:], in_=txtbM[:], func=mybir.ActivationFunctionType.Copy,
    )

    S = pools["sn"].tile([128, NCH], F32, tag="S")
    N = pools["sn"].tile([128, NCH], F32, tag="N")
    for c in range(NCH):
        pj = pools["dve"].tile([128, D], F32, tag="pj")
        nc.vector.scalar_tensor_tensor(
            out=pj[:], in0=x[:, c, :], scalar=1.0, in1=txtbS[:],
            op0=mybir.AluOpType.mult, op1=mybir.AluOpType.mult,
            accum_out=S[:, c : c + 1],
        )
    for c in range(NCH):
        sj = pools["act"].tile([128, D], F32, tag="sj")
        nc.scalar.activation(
            out=sj[:], in_=x[:, c, :],
            func=mybir.ActivationFunctionType.Square,
            accum_out=N[:, c : c + 1],
        )
    st["S"], st["N"] = S, N
    return st


def _strip_out_waw(inst_h, prior_names):
    """Remove sync deps on earlier out-writers. All writers of `out` touch
    provably disjoint rows (CLS rows b*289; batch b's scatters only rows
    [b*289+1, b*289+288], slots unique via cumsum), so the completion-order
    WAW edges Tile inserts between them only serialize the DMA queue."""
    inst = getattr(inst_h, "ins", inst_h)
    for dep in list(inst.sync_dependency_names()):
        if dep in prior_names:
            try:
                inst.try_remove_dependency(dep)
            except Exception:
                inst.remove_dependency(dep)
    prior_names.add(inst.name)
    return inst


def _stage_b(nc, pools, consts, out_flat, i, st, out_writers):
    """u, ranks, destinations, scatter for pair i."""
    b0 = 2 * i
    x, S, N = st["x"], st["S"], st["N"]
    sp = pools["small"]

    # ---- u = sign(s) * s^2 / n  (monotone in the cosine sim) ----
    SS = sp.tile([128, NCH], F32, tag="SS")
    nc.vector.tensor_tensor(out=SS[:], in0=S[:], in1=S[:],
                            op=mybir.AluOpType.mult)
    REC = sp.tile([128, NCH], F32, tag="REC")
    nc.vector.reciprocal(REC[:], N[:])
    UA = sp.tile([128, NCH], F32, tag="UA")
    nc.vector.tensor_tensor(out=UA[:], in0=SS[:], in1=REC[:],
                            op=mybir.AluOpType.mult)
    SGN = sp.tile([128, NCH], I32, tag="SGN")
    nc.vector.tensor_scalar(
        out=SGN[:], in0=S[:].bitcast(I32), scalar1=-0x80000000,
        scalar2=None, op0=mybir.AluOpType.bitwise_and,
    )
    U = sp.tile([128, NCH], F32, tag="U")
    nc.vector.tensor_tensor(
        out=U[:].bitcast(I32), in0=UA[:].bitcast(I32), in1=SGN[:],
        op=mybir.AluOpType.bitwise_or,
    )

    # ---- RB[i, c, j] = u at (partition j + 64*(i>=64), chunk c) ----
    RB = pools["ppr"].tile([128, NCH, H], F32, tag="RB")
    for c in range(NCH):
        rh = pools["dve"].tile([128, H], F32, tag="rh")
        nc.vector.tensor_scalar(
            out=rh[:], in0=consts["D2c"][:],
            scalar1=U[:, c : c + 1], scalar2=None,
            op0=mybir.AluOpType.mult,
        )
        nc.tensor.matmul(
            RB[:, c, :], lhsT=consts["Hc"][:], rhs=rh[:],
            start=True, stop=True,
        )

    # ---- exact ranks within own batch ----
    RANK = sp.tile([128, NCH], F32, tag="RANK")
    for c in range(NCH):
        cj = pools["dve"].tile([128, NCH, H], F32, tag="cj")
        nc.vector.tensor_scalar(
            out=cj[:], in0=RB[:, :, :],
            scalar1=U[:, c : c + 1], scalar2=0.0,
            op0=mybir.AluOpType.is_gt,
            op1=mybir.AluOpType.add,
            accum_out=RANK[:, c : c + 1],
        )

    # ---- keep mask, destination slots ----
    mask = sp.tile([128, NCH], F32, tag="mask")
    nc.vector.tensor_scalar(
        out=mask[:], in0=RANK[:], scalar1=float(K), scalar2=None,
        op0=mybir.AluOpType.is_lt,
    )
    scan = sp.tile([128, NCH], F32, tag="scan")
    nc.vector.tensor_tensor_scan(
        out=scan[:], data0=mask[:], data1=mask[:], initial=0.0,
        op0=mybir.AluOpType.add, op1=mybir.AluOpType.bypass,
    )
    prior = pools["ppp"].tile([128, 1], F32, tag="prior")
    nc.tensor.matmul(prior[:], lhsT=consts["BLT"][:], rhs=scan[:, NCH - 1 : NCH],
                     start=True, stop=True)
    base = sp.tile([128, 1], F32, tag="base")
    nc.vector.tensor_scalar(
        out=base[:], in0=consts["halfvec"][:],
        scalar1=float(OUT_TOK), scalar2=float(b0 * OUT_TOK),
        op0=mybir.AluOpType.mult, op1=mybir.AluOpType.add,
    )
    e = sp.tile([128, 1], F32, tag="e")
    nc.vector.tensor_tensor(out=e[:], in0=prior[:], in1=base[:],
                            op=mybir.AluOpType.add)
    penalty = sp.tile([128, NCH], F32, tag="penalty")
    nc.vector.tensor_scalar(
        out=penalty[:], in0=mask[:], scalar1=-BIG, scalar2=BIG,
        op0=mybir.AluOpType.mult, op1=mybir.AluOpType.add,
    )
    destf = sp.tile([128, NCH], F32, tag="destf")
    nc.vector.scalar_tensor_tensor(
        out=destf[:], in0=scan[:], scalar=e[:, 0:1], in1=penalty[:],
        op0=mybir.AluOpType.add, op1=mybir.AluOpType.add,
    )
    desti = sp.tile([128, NCH], I32, tag="desti")
    nc.vector.tensor_copy(out=desti[:], in_=destf[:])

    # ---- scatter kept rows (one offset column per chunk) ----
    for c in range(NCH):
        h = nc.gpsimd.indirect_dma_start(
            out=out_flat[:, :],
            out_offset=IndirectOffsetOnAxis(ap=desti[:, c : c + 1], axis=0),
            in_=x[:, c, :],
            in_offset=None,
            bounds_check=(b0 + 1) * OUT_TOK + K,
            oob_is_err=False,
        )
        _strip_out_waw(h, out_writers)


def build(nc, b_core=B_CORE, img=None, txt=None, out=None):
    if img is None:
        img = nc.dram_tensor("image_features", [b_core, NUM_TOKENS, D], F32,
                             kind="ExternalInput").ap()
        txt = nc.dram_tensor("text_features", [b_core, D], F32,
                             kind="ExternalInput").ap()
        out = nc.dram_tensor("out", [b_core, OUT_TOK, D], F32,
                             kind="ExternalOutput").ap()

    out_flat = out.rearrange("b k d -> (b k) d")
    npair = b_core // 2

    with tile.TileContext(nc) as tc:
        with (
            tc.tile_pool(name="consts", bufs=1) as cpool,
            tc.tile_pool(name="x", bufs=LAG + 1) as xpool,
            tc.tile_pool(name="sn", bufs=LAG + 1) as snpool,
            tc.tile_pool(name="small", bufs=2) as spool,
            tc.tile_pool(name="dve", bufs=3) as dvepool,
            tc.tile_pool(name="act", bufs=2) as actpool,
            tc.tile_pool(name="gp", bufs=2) as gppool,
            tc.tile_pool(name="txt", bufs=2) as txtpool,
            tc.tile_pool(name="ps_txt", bufs=2, space="PSUM") as ppt,
            tc.tile_pool(name="ps_rb", bufs=1, space="PSUM") as ppr,
            tc.tile_pool(name="ps_pr", bufs=2, space="PSUM") as ppp,
        ):
            # D2c = vstack(I64, I64): places u of partitions j and j+64 in col j
            D2c = cpool.tile([128, H], F32)
            nc.gpsimd.memset(D2c[:], 0.0)
            nc.gpsimd.affine_select(
                out=D2c[:], in_=D2c[:],
                compare_op=mybir.AluOpType.not_equal, fill=1.0,
                base=0, pattern=[[-1, H]], channel_multiplier=1,
            )
            nc.gpsimd.affine_select(
                out=D2c[:], in_=D2c[:],
                compare_op=mybir.AluOpType.not_equal, fill=1.0,
                base=-H, pattern=[[-1, H]], channel_multiplier=1,
            )
            # Hc[k, i] = 1 iff k and i are in the same half
            Hc = cpool.tile([128, 128], F32)
            nc.vector.memset(Hc[:], 0.0)
            nc.vector.memset(Hc[0:H, 0:H], 1.0)
            nc.vector.memset(Hc[H:128, H:128], 1.0)
            # HS rows 0/1 = the half-selector lhsT: HS[0, i] = (i < 64),
            # HS[1, i] = (i >= 64). Rows 2+ are never read. Built on a full
            # [128, 128] tile: sub-partition memsets and small-partition
            # affine_selects are rejected by the compiler.
            HS = cpool.tile([128, 128], F32)
            nc.gpsimd.memset(HS[:], 1.0)
            # zero row 0 where i >= 64: iota = 63 - i + 200k
            nc.gpsimd.affine_select(
                out=HS[:], in_=HS[:],
                compare_op=mybir.AluOpType.is_ge, fill=0.0,
                base=H - 1, pattern=[[-1, 128]], channel_multiplier=200,
            )
            # zero row 1 where i < 64: iota = i + 136 - 200k
            nc.gpsimd.affine_select(
                out=HS[:], in_=HS[:],
                compare_op=mybir.AluOpType.is_ge, fill=0.0,
                base=200 - H, pattern=[[1, 128]], channel_multiplier=-200,
            )
            # halfvec[p] = 1 iff p >= 64
            halfvec = cpool.tile([128, 1], F32)
            nc.vector.memset(halfvec[0:H], 0.0)
            nc.vector.memset(halfvec[H:128], 1.0)
            # BLT[k, i] = 1 iff k < i and same half (cross-partition prefix)
            BLT = cpool.tile([128, 128], F32)
            make_upper_triangular(nc, BLT[:], val=1.0, diag=False)
            nc.gpsimd.memset(BLT[0:H, H:128], 0.0)

            consts = {"D2c": D2c, "Hc": Hc, "HS": HS,
                      "halfvec": halfvec, "BLT": BLT}
            pools = {"x": xpool, "sn": snpool, "small": spool,
                     "dve": dvepool, "act": actpool, "gp": gppool,
                     "txt": txtpool, "ppt": ppt, "ppr": ppr, "ppp": ppp}

            # CLS passthrough for all batches (SBUF bounce).
            out_writers = set()
            clsbuf = cpool.tile([b_core, D], F32)
            nc.sync.dma_start(out=clsbuf[:], in_=img[:, 0, :])
            hcls = nc.sync.dma_start(out=out[:, 0, :], in_=clsbuf[:])
            out_writers.add(getattr(hcls, "ins", hcls).name)

            states = {}
            for i in range(npair + LAG):
                if i < npair:
                    states[i] = _stage_a(nc, pools, consts, img, txt, i)
                j = i - LAG
                if j >= 0:
                    _stage_b(nc, pools, consts, out_flat, j, states.pop(j),
                             out_writers)
    return nc


_CACHED = {}


def _get_nc():
    if "nc" not in _CACHED:
        nc = bacc.Bacc("TRN2", target_bir_lowering=False)
        build(nc)
        nc.compile()
        _CACHED["nc"] = nc
    return _CACHED["nc"]


LAST_RESULT = None


def kernel(image_features, text_features):
    global LAST_RESULT
    from concourse.bass_utils import run_bass_kernel_spmd

    img = np.ascontiguousarray(np.asarray(image_features, dtype=np.float32))
    txt = np.ascontiguousarray(np.asarray(text_features, dtype=np.float32))
    assert img.shape == (B_FULL, NUM_TOKENS, D)
    assert txt.shape == (B_FULL, D)

    nc = _get_nc()
    in_maps = [
        {
            "image_features": img[i * B_CORE : (i + 1) * B_CORE],
            "text_features": txt[i * B_CORE : (i + 1) * B_CORE],
        }
        for i in range(N_CORES)
    ]
    res = run_bass_kernel_spmd(nc, in_maps, core_ids=list(range(N_CORES)))
    LAST_RESULT = res
    return np.concatenate([res.results[i]["out"] for i in range(N_CORES)], axis=0)
